# revision 15
# baseline (speedup 1.0000x reference)
"""Trainium2 Bass kernel for nn_FCGAT (fully-connected GAT block).

Math: the reference computes
    h      = x @ W + bW
    scores = LeakyReLU(s_i[:,None] + s_j[None,:] + a_b)
    a      = softmax(scores, axis=-1)
    out    = relu(einsum('nkj,nkd->nkd', a, h))
The einsum contracts `a` over j only, i.e. multiplies h elementwise by the
softmax row-sums, which are exactly 1.  So out == relu(x @ W + bW) up to
float rounding (verified: scale-relative absmax ~1e-6 vs the jax reference).
The kernel therefore runs a memory-bound fused GEMM+bias+relu, data-parallel
over the batch dim N across 8 NeuronCores.

Device layout (per core, rows = 8*1024 = 8192):
  The host hands each core its x shard transposed (xT: [128 feat, 8192 rows])
  so the contraction dim lands on SBUF partitions with no on-device
  transposes.  W stays stationary in the PE array; each matmul streams 512
  rows as the moving operand into one PSUM bank, producing h^T.  In this
  transposed layout the bias is per-partition, so ONE scalar-engine
  activation per matmul fuses bias + relu + PSUM->SBUF.  The output (out^T)
  is DMA'd back and un-transposed on the host while unsharding.
"""

import os

import numpy as np

import concourse.bacc as bacc
import concourse.mybir as mybir
import concourse.tile as tile
from concourse.bass_utils import run_bass_kernel_spmd

N, K, D1, D2 = 64, 1024, 128, 128
NCORES = 8
ROWS = (N // NCORES) * K  # 8192 rows per core
CH = 2048  # rows per DMA chunk (1 MiB)
NCH = ROWS // CH  # 4 chunks
MM = 512  # moving rows per fp32 matmul (= one PSUM bank)

F32 = mybir.dt.float32

_nc_cache = None

# Results of the most recent hardware run (BassKernelResults); lets a test
# harness read exec_time_ns when KERNEL_TRACE=1 is set.
LAST_RESULTS = None


def _build_nc(repeat=1):
    """Build the per-core Bass kernel.

    ``repeat`` re-runs the identical pipeline that many times inside one
    NEFF (same DRAM in/out) — used only for slope-based HW timing.
    """
    nc = bacc.Bacc("TRN2", target_bir_lowering=False, debug=False)

    xt = nc.dram_tensor("xT", [D1, ROWS], F32, kind="ExternalInput").ap()
    w = nc.dram_tensor("W", [D1, D2], F32, kind="ExternalInput").ap()
    bw = nc.dram_tensor("bW", [D2, 1], F32, kind="ExternalInput").ap()
    outt = nc.dram_tensor("outT", [D2, ROWS], F32, kind="ExternalOutput").ap()

    with tile.TileContext(nc) as tc:
        with (
            tc.tile_pool(name="const", bufs=1) as cpool,
            tc.tile_pool(name="xin", bufs=3) as xpool,
            tc.tile_pool(name="oout", bufs=3) as opool,
            tc.tile_pool(name="ps", bufs=4, space="PSUM") as pspool,
            tc.tile_pool(name="warm", bufs=1, space="PSUM") as wpool,
        ):
            # Constants go over SWDGE (gpsimd): tiny transfers on their own
            # queues, so the SP HWDGE ring starts streaming x immediately.
            w_s = cpool.tile([D1, D2], F32)
            nc.gpsimd.dma_start(w_s[:], w)
            bias_s = cpool.tile([D2, 1], F32)
            nc.gpsimd.dma_start(bias_s[:], bw)

            # PE warm-up: ~3.4us of chained dummy matmuls on zeros releases
            # the HAM clock throttle before the first real matmul arrives.
            # The dummy activation forces the Relu table load off the
            # critical path.
            warm = cpool.tile([D1, 256], F32)
            nc.gpsimd.memset(warm[:], 0.0)
            nc.scalar.activation(
                warm[:], warm[:], mybir.ActivationFunctionType.Relu, bias=0.0
            )
            wps = wpool.tile([D2, 256], F32)
            NWARM = 4
            for i in range(NWARM):
                nc.tensor.matmul(
                    wps[:],
                    lhsT=warm[:, :D2],
                    rhs=warm[:],
                    start=(i == 0),
                    stop=(i == NWARM - 1),
                )

            # smaller first/last chunks shrink pipeline head/tail
            chunk_sizes = [CH // 2] + [CH] * (NCH - 1) + [CH // 2]
            for _r in range(repeat):
                pos = 0
                for ci, csz in enumerate(chunk_sizes):
                    xin = xpool.tile([D1, CH], F32, tag="xin")
                    # loads on the SP HWDGE ring
                    nc.sync.dma_start(xin[:, :csz], xt[:, pos : pos + csz])
                    oout = opool.tile([D2, CH], F32, tag="oout")
                    for m in range(csz // MM):
                        ps = pspool.tile([D2, MM], F32, tag="ps")
                        nc.tensor.matmul(
                            ps[:],
                            lhsT=w_s[:],
                            rhs=xin[:, m * MM : (m + 1) * MM],
                            start=True,
                            stop=True,
                        )
                        nc.scalar.activation(
                            oout[:, m * MM : (m + 1) * MM],
                            ps[:],
                            mybir.ActivationFunctionType.Relu,
                            bias=bias_s[:],
                        )
                    # stores on SWDGE queues: they never queue behind the
                    # loads on the SP HWDGE ring.  The LAST store instead goes
                    # over the SP HWDGE ring (idle by then, lower fixed
                    # latency) to shorten the kernel tail.
                    if ci == len(chunk_sizes) - 1:
                        nc.sync.dma_start(outt[:, pos : pos + csz], oout[:, :csz])
                    else:
                        nc.gpsimd.dma_start(outt[:, pos : pos + csz], oout[:, :csz])
                    pos += csz

    nc.compile()
    return nc


def kernel(x, W, bW, a_w=None, a_b=None, **_unused):
    global _nc_cache, LAST_RESULTS
    if _nc_cache is None:
        _nc_cache = _build_nc()
    nc = _nc_cache

    x_flat = np.asarray(x, dtype=np.float32).reshape(N * K, D1)
    W_ = np.ascontiguousarray(np.asarray(W, dtype=np.float32))
    bW_ = np.ascontiguousarray(np.asarray(bW, dtype=np.float32).reshape(D2, 1))

    in_maps = []
    for i in range(NCORES):
        shard_t = np.ascontiguousarray(x_flat[i * ROWS : (i + 1) * ROWS].T)
        in_maps.append({"xT": shard_t, "W": W_, "bW": bW_})

    trace = bool(os.environ.get("KERNEL_TRACE"))
    try:
        res = run_bass_kernel_spmd(nc, in_maps, list(range(NCORES)), trace=trace)
    except ModuleNotFoundError:
        # Chipless axon client without the NTFF profile hook package —
        # rerun without tracing.
        os.environ["BASS_NEVER_TRACE"] = "1"
        res = run_bass_kernel_spmd(nc, in_maps, list(range(NCORES)), trace=False)
    LAST_RESULTS = res

    out = np.concatenate(
        [np.asarray(res.results[i]["outT"]).T for i in range(NCORES)], axis=0
    )
    return np.ascontiguousarray(out.reshape(N, K, D2))
